# revision 81
# baseline (speedup 1.0000x reference)
"""Trainium2 Bass kernel for nn_AutoregressiveLSA.

Reference math (complex, per batch b):
    Q  = WKQ @ E                      [2d, T]
    S  = E^H @ Q, keep i <= j         [T, T]
    out= WPV @ (E @ S) / rho_j        [d, T], cols 1..T-2 returned

Re-associated as out = (WPV @ E) @ S, and computed transposed:
    PT[t, d] = (WPV @ E)^T            (lhsT = E, rhs = WPV^T)
    outT[j,d] = sum_{i<=j} S[i,j] PT[i,d] / rho_j

Sharding: data-parallel over batch, one NeuronCore per batch element.
All matmuls run as float32r (FP22 mantissa-truncated fp32) with free dim
>= 256, which streams at 1 column/cycle on the PE. The walrus verifier
requires f32r matmul operands to come from f32r producers, so every
matmul operand tile AND the external inputs feeding them are declared
f32r end-to-end (raw f32 bits are shipped; the PE truncates).

All four complex matmul phases use Karatsuba (3 real matmuls per
complex product) with the recombination folded into the PSUM-evacuation
vector ops. The operand sums/differences are host-prepared where free
(WKQ^T variants, E_re+E_im) and computed on DVE where cheap (per-block
slabs), keeping the tensor engine the only near-saturated unit.
"""

import numpy as np

import concourse.bass as bass
import concourse.mybir as mybir
import concourse.tile as tile
from concourse import bacc
from concourse.bass_utils import run_bass_kernel_spmd

F32 = mybir.dt.float32
F32R = mybir.dt.float32r

# Problem dims (hardcoded per contract)
B = 8
D2 = 1024   # 2*dim, channel dim of E
T = 2048    # sequence length
D = 512     # output channel dim
P = 128


def _mm(nc, out, lhsT, rhs, start, stop):
    nc.tensor.matmul(out, lhsT, rhs, start=start, stop=stop)


def build_module(D2=D2, T=T, D=D, QPAN=512, SPAN=256):
    """Build the per-core Bass module (same NEFF on all 8 cores)."""
    KC = D2 // P      # k-tiles over channel dim
    MB = D2 // P      # m-tiles for Q rows
    TB = T // P       # blocks over sequence
    NQP = T // QPAN   # Q panels (phase A1)
    NSP = T // SPAN   # S panels (phase B)
    nc = bacc.Bacc(target_bir_lowering=False, trn_type="TRN2")

    e_re = nc.dram_tensor("e_re", [D2, T], F32R, kind="ExternalInput")
    e_im = nc.dram_tensor("e_im", [D2, T], F32R, kind="ExternalInput")
    e_sum = nc.dram_tensor("e_sum", [D2, T], F32R, kind="ExternalInput")
    wt_re = nc.dram_tensor("wt_re", [MB, P, KC, P], F32R, kind="ExternalInput")
    wt_im = nc.dram_tensor("wt_im", [MB, P, KC, P], F32R, kind="ExternalInput")
    wt_sum = nc.dram_tensor("wt_sum", [MB, P, KC, P], F32R, kind="ExternalInput")
    wv_re = nc.dram_tensor("wv_re", [D2, D], F32R, kind="ExternalInput")
    wv_im = nc.dram_tensor("wv_im", [D2, D], F32R, kind="ExternalInput")
    trimask = nc.dram_tensor("trimask", [P, P], F32R, kind="ExternalInput")
    rho = nc.dram_tensor("rho", [P, TB], F32, kind="ExternalInput")
    outT_re = nc.dram_tensor("outT_re", [T, D], F32, kind="ExternalOutput")
    outT_im = nc.dram_tensor("outT_im", [T, D], F32, kind="ExternalOutput")

    with tile.TileContext(nc) as tc:
        with tc.tile_pool(name="dram", bufs=1, space="DRAM") as dram, \
             tc.tile_pool(name="ps", bufs=2, space="PSUM") as ps, \
             tc.tile_pool(name="ev", bufs=4) as ev, \
             tc.tile_pool(name="early", bufs=1) as early:
            q_re = dram.tile([MB, P, T], F32R, tag="q_re")
            q_im = dram.tile([MB, P, T], F32R, tag="q_im")
            pt_re = dram.tile([TB, P, D], F32R, tag="pt_re")
            pt_im = dram.tile([TB, P, D], F32R, tag="pt_im")
            s_re = dram.tile([TB, P, T], F32R, tag="s_re")
            s_im = dram.tile([TB, P, T], F32R, tag="s_im")
            se_re = [dram.tile([P, (jb + 1) * P], F32R, tag=f"se_re{jb}", name=f"se_re{jb}")
                     for jb in range(2)]
            se_im = [dram.tile([P, (jb + 1) * P], F32R, tag=f"se_im{jb}", name=f"se_im{jb}")
                     for jb in range(2)]

            _ctr = [0]

            def psum3(width):
                # round-robin 3 accumulators over 4 tags x 2 bufs = all
                # 8 PSUM banks, deepening the accumulate->evac pipeline
                _ctr[0] += 1
                n = _ctr[0]
                t = [f"p{(3 * n + k) % 4}" for k in range(3)]
                return (ps.tile([P, QPAN], F32, tag=t[0], name=f"pa{n}")[:, :width],
                        ps.tile([P, QPAN], F32, tag=t[1], name=f"pb{n}")[:, :width],
                        ps.tile([P, QPAN], F32, tag=t[2], name=f"pc{n}")[:, :width])

            def psum2(width):
                _ctr[0] += 1
                n = _ctr[0]
                t = [f"p{(3 * n + k) % 4}" for k in range(2)]
                return (ps.tile([P, QPAN], F32, tag=t[0], name=f"pa{n}")[:, :width],
                        ps.tile([P, QPAN], F32, tag=t[1], name=f"pc{n}")[:, :width])

            def ev2(width, dtype=F32R):
                _ctr[0] += 1
                n = _ctr[0]
                return (ev.tile([P, QPAN], dtype, tag="ev0", name=f"ev0_{n}")[:, :width],
                        ev.tile([P, QPAN], dtype, tag="ev1", name=f"ev1_{n}")[:, :width])

            # Early-reserved phase-C tiles: outside E's address range, so
            # their DMAs run during B instead of waiting for E to free.
            N_EPT = 2   # PT blocks preloaded early
            N_EJB = 2   # first C iterations fully early-staged
            ptr_e = [early.tile([P, D], F32R, tag=f"ptre{i}", name=f"ptre{i}")
                     for i in range(N_EPT)]
            pti_e = [early.tile([P, D], F32R, tag=f"ptie{i}", name=f"ptie{i}")
                     for i in range(N_EPT)]
            srs_e = [early.tile([P, jb + 1, P], F32R, tag=f"srse{jb}", name=f"srse{jb}")
                     for jb in range(N_EJB)]
            sis_e = [early.tile([P, jb + 1, P], F32R, tag=f"sise{jb}", name=f"sise{jb}")
                     for jb in range(N_EJB)]
            rho_sb = early.tile([P, TB], F32, tag="rho")
            nc.gpsimd.dma_start(rho_sb[:], rho[:])

            # ---- Phase A1 (first): Q = WKQ @ E -> DRAM ----
            # Karatsuba: M1=WTr.Er M2=WTi.Ei M3=(WTr+WTi)(Er+Ei)
            # Qr = M1-M2, Qi = M3-M1-M2.  WKQ^T resident (3 variants),
            # E streamed as 256-wide panels (re/im on HWDGE, sum via host).
            A1P = 256
            with tc.tile_pool(name="wtres", bufs=1) as wtrp, \
                 tc.tile_pool(name="epan", bufs=2) as epp:
                wtres_r = wtrp.tile([P, KC, MB * P], F32R, tag="wtres_r")
                wtres_i = wtrp.tile([P, KC, MB * P], F32R, tag="wtres_i")
                wtres_s = wtrp.tile([P, KC, MB * P], F32R, tag="wtres_s")
                for m in range(MB):
                    mbs = bass.ts(m, P)
                    nc.gpsimd.dma_start(wtres_r[:, :, mbs], wt_re[m])
                    nc.gpsimd.dma_start(wtres_i[:, :, mbs], wt_im[m])
                    nc.gpsimd.dma_start(wtres_s[:, :, mbs], wt_sum[m])
                for jp in range(T // A1P):
                    js = bass.ds(jp * A1P, A1P)
                    er_p = epp.tile([P, KC, A1P], F32R, tag="er_p")
                    ei_p = epp.tile([P, KC, A1P], F32R, tag="ei_p")
                    es_p = epp.tile([P, KC, A1P], F32R, tag="es_p")
                    for kc in range(KC):
                        nc.sync.dma_start(er_p[:, kc], e_re[bass.ts(kc, P), js])
                        nc.sync.dma_start(ei_p[:, kc], e_im[bass.ts(kc, P), js])
                        nc.sync.dma_start(es_p[:, kc], e_sum[bass.ts(kc, P), js])
                    for m in range(MB):
                        mbs = bass.ts(m, P)
                        pa, pb, pc = psum3(A1P)
                        for kc in range(KC):
                            first, last = kc == 0, kc == KC - 1
                            _mm(nc, pa, wtres_r[:, kc, mbs], er_p[:, kc], first, last)
                            _mm(nc, pb, wtres_i[:, kc, mbs], ei_p[:, kc], first, last)
                            _mm(nc, pc, wtres_s[:, kc, mbs], es_p[:, kc], first, last)
                        qr_sb, qi_sb = ev2(A1P)
                        nc.any.tensor_copy(out=qr_sb[:], in_=pa[:])
                        nc.any.tensor_copy(out=qi_sb[:], in_=pc[:])
                        nc.vector.tensor_sub(qi_sb[:], qi_sb[:], pa[:])
                        nc.vector.tensor_sub(qr_sb[:], qr_sb[:], pb[:])
                        nc.vector.tensor_sub(qi_sb[:], qi_sb[:], pb[:])
                        nc.sync.dma_start(q_re[m, :, js], qr_sb[:])
                        nc.gpsimd.dma_start(q_im[m, :, js], qi_sb[:])

            with tc.tile_pool(name="eres", bufs=1) as eres:
                er = eres.tile([P, KC, T], F32R, tag="er")
                ei = eres.tile([P, KC, T], F32R, tag="ei")
                # Load E in phase-A2 consumption order (column chunks of
                # 256), staged: 2 chunks up front, the rest issued inside
                # A2's loop so PT evacuations interleave on the DMA queue.
                def load_e_chunk(tc2):
                    js = bass.ds(tc2 * 256, 256)
                    for kc in range(KC):
                        nc.sync.dma_start(er[:, kc, js], e_re[bass.ts(kc, P), js])
                        nc.gpsimd.dma_start(ei[:, kc, js], e_im[bass.ts(kc, P), js])

                NCH = T // 256

                with tc.tile_pool(name="mask", bufs=1) as mkp:
                    mask_sb = mkp.tile([P, P], F32R, tag="mask")
                    nc.gpsimd.dma_start(mask_sb[:], trimask[:])

                    # ---- Phase A2: PT = (WPV @ E)^T -> DRAM ----
                    # Karatsuba: E^T*WV: M1=Er.WVr M2=Ei.WVi M3=(Er+Ei)(WVr+WVi)
                    # Re = M1-M2, Im = M3-M1-M2
                    with tc.tile_pool(name="wv", bufs=1) as wvp, \
                         tc.tile_pool(name="esum", bufs=1) as esp:
                        # wv loads on the SWDGE queue, parallel to E on HWDGE
                        wvr = wvp.tile([P, KC, D], F32R, tag="wvr")
                        wvi = wvp.tile([P, KC, D], F32R, tag="wvi")
                        wvs = wvp.tile([P, KC, D], F32R, tag="wvs")
                        nc.gpsimd.dma_start(wvr[:], wv_re[:].rearrange("(kc p) d -> p kc d", p=P))
                        nc.gpsimd.dma_start(wvi[:], wv_im[:].rearrange("(kc p) d -> p kc d", p=P))
                        nc.vector.tensor_add(wvs[:], wvr[:], wvi[:])
                        load_e_chunk(0)
                        load_e_chunk(1)
                        for tb in range(TB):
                            if tb % 2 == 0 and tb // 2 + 2 < NCH:
                                load_e_chunk(tb // 2 + 2)
                            ts_ = bass.ts(tb, P)
                            es = esp.tile([P, KC, P], F32R, tag="es", name=f"es{tb}")
                            nc.vector.tensor_add(es[:], er[:, :, ts_], ei[:, :, ts_])
                            pa, pb, pc = psum3(D)
                            for kc in range(KC):
                                first, last = kc == 0, kc == KC - 1
                                _mm(nc, pa, er[:, kc, ts_], wvr[:, kc], first, last)
                                _mm(nc, pb, ei[:, kc, ts_], wvi[:, kc], first, last)
                                _mm(nc, pc, es[:, kc], wvs[:, kc], first, last)
                            ptr_sb, pti_sb = ev2(D)
                            nc.any.tensor_copy(out=ptr_sb[:], in_=pa[:])
                            nc.any.tensor_copy(out=pti_sb[:], in_=pc[:])
                            nc.vector.tensor_sub(pti_sb[:], pti_sb[:], pa[:])
                            nc.vector.tensor_sub(ptr_sb[:], ptr_sb[:], pb[:])
                            nc.vector.tensor_sub(pti_sb[:], pti_sb[:], pb[:])
                            nc.sync.dma_start(pt_re[tb], ptr_sb[:])
                            nc.sync.dma_start(pt_im[tb], pti_sb[:])
                    # Prefetch first PT blocks for phase C (runs during B).
                    for i in range(N_EPT):
                        nc.gpsimd.dma_start(ptr_e[i][:], pt_re[i])
                        nc.gpsimd.dma_start(pti_e[i][:], pt_im[i])

                    # ---- Phase B: S = E^H Q (upper-tri row blocks) -> DRAM ----
                    # Karatsuba: conj(E)*Q per block: M1=Er.Qr  M2=Ei.Qi
                    # M3=(Er-Ei).(Qr+Qi);  Sr=M1+M2, Si=M3-M1+M2
                    with tc.tile_pool(name="qpan", bufs=2) as qp, \
                         tc.tile_pool(name="qsum", bufs=1) as qsp, \
                         tc.tile_pool(name="ediff", bufs=3) as edp:
                        for sp in range(NSP):
                            js = bass.ds(sp * SPAN, SPAN)
                            qr_pan = qp.tile([P, MB, SPAN], F32R, tag="qr_pan")
                            qi_pan = qp.tile([P, MB, SPAN], F32R, tag="qi_pan")
                            nc.sync.dma_start(qr_pan[:], q_re[:, :, js].rearrange("m p t -> p m t"))
                            nc.sync.dma_start(qi_pan[:], q_im[:, :, js].rearrange("m p t -> p m t"))
                            qs_pan = qsp.tile([P, MB, SPAN], F32R, tag="qs_pan")
                            nc.vector.tensor_add(qs_pan[:], qr_pan[:], qi_pan[:])
                            jb_hi = ((sp + 1) * SPAN) // P - 1
                            for ib in range(jb_hi + 1):
                                ibs = bass.ts(ib, P)
                                ed = edp.tile([P, KC, P], F32R, tag="ed", name=f"ed{sp}_{ib}")
                                nc.vector.tensor_sub(ed[:], er[:, :, ibs], ei[:, :, ibs])
                                pa, pb, pc = psum3(SPAN)
                                for kc in range(KC):
                                    first, last = kc == 0, kc == KC - 1
                                    _mm(nc, pa, er[:, kc, ibs], qr_pan[:, kc], first, last)
                                    _mm(nc, pb, ei[:, kc, ibs], qi_pan[:, kc], first, last)
                                    _mm(nc, pc, ed[:, kc], qs_pan[:, kc], first, last)
                                sr_sb, si_sb = ev2(SPAN)
                                nc.any.tensor_copy(out=sr_sb[:], in_=pa[:])
                                nc.vector.tensor_add(sr_sb[:], sr_sb[:], pb[:])
                                nc.any.tensor_copy(out=si_sb[:], in_=pc[:])
                                nc.vector.tensor_sub(si_sb[:], si_sb[:], pa[:])
                                nc.vector.tensor_add(si_sb[:], si_sb[:], pb[:])
                                if ib * P >= sp * SPAN:  # diagonal block inside panel
                                    off = ib * P - sp * SPAN
                                    dsl = bass.ds(off, P)
                                    nc.vector.tensor_mul(sr_sb[:, dsl], sr_sb[:, dsl], mask_sb[:])
                                    nc.vector.tensor_mul(si_sb[:, dsl], si_sb[:, dsl], mask_sb[:])
                                nc.sync.dma_start(s_re[ib, :, js], sr_sb[:])
                                nc.sync.dma_start(s_im[ib, :, js], si_sb[:])
                                # stage first C strips via side buffers so
                                # they do not depend on all of s_re
                                for jb in range(max(ib, sp * SPAN // P),
                                                min(N_EJB, (sp + 1) * SPAN // P)):
                                    jloc = bass.ds(jb * P - sp * SPAN, P)
                                    ioff = bass.ds(ib * P, P)
                                    nc.gpsimd.dma_start(se_re[jb][:, ioff], sr_sb[:, jloc])
                                    nc.gpsimd.dma_start(se_im[jb][:, ioff], si_sb[:, jloc])
                            for jb in range(sp * SPAN // P,
                                            min(N_EJB, (sp + 1) * SPAN // P)):
                                nc.gpsimd.dma_start(
                                    srs_e[jb][:],
                                    se_re[jb][:].rearrange("p (ib j) -> p ib j", j=P))
                                nc.gpsimd.dma_start(
                                    sis_e[jb][:],
                                    se_im[jb][:].rearrange("p (ib j) -> p ib j", j=P))

            # E freed. ---- Phase C: outT[j,d] = sum_{i<=j} S[i,j] PT[i,d] / rho ----
            with tc.tile_pool(name="ptres", bufs=1) as ptp, \
                 tc.tile_pool(name="strip", bufs=3) as stp:
                ptr_t = list(ptr_e)
                pti_t = list(pti_e)
                for tb in range(N_EPT, TB):
                    ptr_t.append(ptp.tile([P, D], F32R, tag=f"ptr{tb}", name=f"ptr{tb}"))
                    pti_t.append(ptp.tile([P, D], F32R, tag=f"pti{tb}", name=f"pti{tb}"))
                    nc.gpsimd.dma_start(ptr_t[tb][:], pt_re[tb])
                    nc.gpsimd.dma_start(pti_t[tb][:], pt_im[tb])
                pts_t = []
                for tb in range(TB):
                    pts_t.append(ptp.tile([P, D], F32R, tag=f"pts{tb}", name=f"pts{tb}"))
                    nc.vector.tensor_add(pts_t[tb][:], ptr_t[tb][:], pti_t[tb][:])
                for jb in range(TB):
                    jbs = bass.ts(jb, P)
                    if jb < N_EJB:
                        srs, sis = srs_e[jb], sis_e[jb]
                    else:
                        srs = stp.tile([P, TB, P], F32R, tag="srs", name=f"srs{jb}")[:, : jb + 1]
                        sis = stp.tile([P, TB, P], F32R, tag="sis", name=f"sis{jb}")[:, : jb + 1]
                        nc.sync.dma_start(
                            srs[:], s_re[: jb + 1, :, jbs].rearrange("ib p j -> p ib j"))
                        nc.sync.dma_start(
                            sis[:], s_im[: jb + 1, :, jbs].rearrange("ib p j -> p ib j"))
                    sss = stp.tile([P, TB, P], F32R, tag="sss", name=f"sss{jb}")[:, : jb + 1]
                    nc.vector.tensor_add(sss[:], srs[:], sis[:])
                    # Karatsuba: M1=Sr·PTr  M2=Si·PTi  M3=(Sr+Si)(PTr+PTi)
                    # Re = M1-M2, Im = M3-M1-M2
                    pa, pb, pc = psum3(D)
                    for ib in range(jb + 1):
                        first, last = ib == 0, ib == jb
                        _mm(nc, pa, srs[:, ib], ptr_t[ib][:], first, last)
                        _mm(nc, pb, sis[:, ib], pti_t[ib][:], first, last)
                        _mm(nc, pc, sss[:, ib], pts_t[ib][:], first, last)
                    our, oui = ev2(D, F32)
                    rb = rho_sb[:, jb : jb + 1].to_broadcast([P, D])
                    nc.any.tensor_copy(out=our[:], in_=pa[:])
                    nc.any.tensor_copy(out=oui[:], in_=pc[:])
                    nc.vector.tensor_sub(oui[:], oui[:], our[:])
                    nc.vector.tensor_sub(oui[:], oui[:], pb[:])
                    nc.vector.tensor_sub(our[:], our[:], pb[:])
                    nc.vector.tensor_mul(our[:], our[:], rb)
                    nc.vector.tensor_mul(oui[:], oui[:], rb)
                    nc.sync.dma_start(outT_re[jbs, :], our[:])
                    nc.sync.dma_start(outT_im[jbs, :], oui[:])

    nc.compile()
    return nc


_NC_CACHE = None


def _get_module():
    global _NC_CACHE
    if _NC_CACHE is None:
        _NC_CACHE = build_module()
    return _NC_CACHE


def prep_shared(WKQ_re, WKQ_im, WPV_re, WPV_im, D2=D2, T=T):
    """Host-side weight prep, shared across cores."""
    KC = D2 // P
    MB = D2 // P
    TB = T // P

    def blk(w):  # WKQ^T blocked for per-m lhsT streaming
        wt = np.ascontiguousarray(w.T)            # [c, c']
        return np.ascontiguousarray(
            wt.reshape(KC, P, MB, P).transpose(2, 1, 0, 3))  # [m, p, kc, w]

    shared = {
        "wt_re": blk(WKQ_re),
        "wt_im": blk(WKQ_im),
        "wt_sum": blk(WKQ_re + WKQ_im),
        "wv_re": np.ascontiguousarray(WPV_re.T),  # [c, d]
        "wv_im": np.ascontiguousarray(WPV_im.T),
        "trimask": np.triu(np.ones((P, P), np.float32)),
    }
    j = np.arange(T, dtype=np.float32)
    rho = 1.0 / np.maximum(j, 1.0)
    shared["rho"] = np.ascontiguousarray(
        rho.reshape(TB, P).T)  # [p, jb]
    return shared


def kernel(E_re, E_im, WKQ_re, WKQ_im, WPV_re, WPV_im):
    E_re = np.asarray(E_re, dtype=np.float32)
    E_im = np.asarray(E_im, dtype=np.float32)
    shared = prep_shared(np.asarray(WKQ_re, np.float32),
                         np.asarray(WKQ_im, np.float32),
                         np.asarray(WPV_re, np.float32),
                         np.asarray(WPV_im, np.float32))
    in_maps = []
    for b in range(B):
        m = dict(shared)
        m["e_re"] = np.ascontiguousarray(E_re[b])
        m["e_im"] = np.ascontiguousarray(E_im[b])
        m["e_sum"] = E_re[b] + E_im[b]
        in_maps.append(m)

    nc = _get_module()
    res = run_bass_kernel_spmd(nc, in_maps, core_ids=list(range(B)))

    out = np.empty((B, D, T - 2), dtype=np.complex64)
    for b in range(B):
        r = res.results[b]["outT_re"]  # [T, D]
        i = res.results[b]["outT_im"]
        full = (r + 1j * i.astype(np.complex64)).T  # [D, T]
        out[b] = full[:, 1 : T - 1]
    return out

